# revision 11
# baseline (speedup 1.0000x reference)
"""Trainium2 Bass kernel for sliding-window GQA attention (nn_Attention_12610023981270).

Sharding: 8 cores, head-parallel — core i owns q-heads {2i, 2i+1} and kv-head i
for projections + attention, then per-head AllToAlls switch to sequence-parallel
for the output projection (core i produces output rows [256*i, 256*(i+1))).

Everything on-chip stays "transposed" ([feature, token]) so the only PE
transposes needed are x itself and a small v fix-up.  Matmuls run in float32r
(full-rate fp32 PE mode, ~1.6e-4 rel-err) with 512-wide moving operands to
amortize the un-hidden fp32r LDWEIGHTS cost.

Model: B=1, T=2048, D=3584, 16 q-heads / 8 kv-heads, head_dim 256,
RoPE, query_scale 1/16, logit softcap 50, causal + sliding window 1024.
"""
import sys

if '/opt/trn_rl_repo' not in sys.path:
    sys.path.insert(0, '/opt/trn_rl_repo')

import numpy as np

import concourse.bass as bass
import concourse.mybir as mybir
import concourse.tile as tile
from concourse import bacc
from concourse.bass_utils import run_bass_kernel_spmd

f32 = mybir.dt.float32
f32r = mybir.dt.float32r
i32 = mybir.dt.int32
AF = mybir.ActivationFunctionType
Alu = mybir.AluOpType

N_CORES = 8
T, D, HD = 2048, 3584, 256
DC = D // 128            # 28 d-chunks
TWO_PI = 6.283185307179586
HALF_PI = 1.5707963267948966
SOFT_CAP = 50.0
QUERY_SCALAR = 0.0625
WINDOW = 1024
MASK_VAL = -1.0e6
TANH_SCALE = QUERY_SCALAR / SOFT_CAP   # folds query scaling into the softcap

CAUSAL_DD = (0, 128, 256, 384)
WINDOW_DD = (-1024, -896, -768, -640)


def _live_chunks(tb):
    t0 = tb * 512
    smin = max(0, t0 - (WINDOW - 1))
    smax = t0 + 511
    return list(range(smin // 128, smax // 128 + 1))


def _build_module():
    nc = bacc.Bacc("TRN2", target_bir_lowering=False, debug=False,
                   num_devices=N_CORES)

    x_in = nc.declare_dram_parameter("x", [T, D], f32, isOutput=False)
    pos_in = nc.declare_dram_parameter("pos", [1, T], i32, isOutput=False)
    wq_in = nc.declare_dram_parameter("wq", [D, 512], f32, isOutput=False)
    wk_in = nc.declare_dram_parameter("wk", [D, 256], f32, isOutput=False)
    wv_in = nc.declare_dram_parameter("wv", [D, 256], f32, isOutput=False)
    wo_in = nc.declare_dram_parameter("wo", [4096, D], f32, isOutput=False)
    # consts: [:, 0:128] identity, [:, 128] ones, [:, 129] inv_timescale
    consts_in = nc.declare_dram_parameter("consts", [128, 130], f32, isOutput=False)
    out_ext = nc.declare_dram_parameter("out", [T // N_CORES, D], f32, isOutput=True)

    qT_d = nc.dram_tensor("qT_d", [512, T], f32)
    kT_d = nc.dram_tensor("kT_d", [256, T], f32)
    vT_d = nc.dram_tensor("vT_d", [256, T], f32)
    cc_in = [nc.dram_tensor(f"cc_in{h}", [8, 256, 256], f32) for h in range(2)]
    cc_out = [nc.dram_tensor(f"cc_out{h}", [8, 256, 256], f32) for h in range(2)]

    with tile.TileContext(nc) as tc:
        with tc.tile_pool(name="prep", bufs=1) as prep:
            ident_r = prep.tile([128, 128], f32r)
            nc.sync.dma_start(ident_r[:], consts_in[:, 0:128].bitcast(f32r))
            ones_col_r = prep.tile([128, 1], f32r)
            nc.sync.dma_start(ones_col_r[:], consts_in[:, 128:129].bitcast(f32r))
            ones_row_f = prep.tile([1, 128], f32)
            nc.sync.dma_start(ones_row_f[:],
                              consts_in[:, 128:129].rearrange("p one -> one p"))
            inv_ts = prep.tile([128, 1], f32)
            nc.sync.dma_start(inv_ts[:], consts_in[:, 129:130])

            # ---------- phase 0: RoPE sin/cos tables [128, T] ----------
            with tc.tile_pool(name="tables", bufs=1) as tbl:
                sin_t = tbl.tile([128, T], f32)
                cos_t = tbl.tile([128, T], f32)
                with (
                    tc.tile_pool(name="p0", bufs=1) as p0,
                    tc.tile_pool(name="ps0", bufs=2, space="PSUM") as ps0,
                ):
                    pos_i = p0.tile([1, T], i32)
                    nc.sync.dma_start(pos_i[:], pos_in[:])
                    pos_f = p0.tile([1, T], f32)
                    nc.vector.tensor_copy(pos_f[:], pos_i[:])
                    theta = p0.tile([128, T], f32)
                    for b in range(T // 512):
                        ps = ps0.tile([128, 512], f32, tag="bc0")
                        nc.tensor.matmul(ps[:], ones_row_f[:],
                                         pos_f[:, b * 512:(b + 1) * 512],
                                         start=True, stop=True)
                        nc.vector.tensor_scalar(theta[:, b * 512:(b + 1) * 512],
                                                ps[:], inv_ts[:], None, Alu.mult)

                    def range_reduce(dst, pre_add):
                        u = p0.tile([128, T], f32, tag="rr_u")
                        nc.vector.tensor_scalar(u[:], theta[:], pre_add,
                                                1.0 / TWO_PI, Alu.add, Alu.mult)
                        k_i = p0.tile([128, T], i32, tag="rr_k")
                        nc.vector.tensor_copy(k_i[:], u[:])
                        k_f = p0.tile([128, T], f32, tag="rr_kf")
                        nc.vector.tensor_copy(k_f[:], k_i[:])
                        r = p0.tile([128, T], f32, tag="rr_r")
                        nc.vector.tensor_tensor(r[:], u[:], k_f[:], Alu.subtract)
                        nc.vector.tensor_scalar(dst[:], r[:], TWO_PI, None,
                                                Alu.mult)

                    th_r = p0.tile([128, T], f32, tag="th_r")
                    range_reduce(th_r, 0.0)
                    nc.scalar.activation(sin_t[:], th_r[:], AF.Sin)
                    th_r2 = p0.tile([128, T], f32, tag="th_r")
                    range_reduce(th_r2, HALF_PI)
                    nc.scalar.activation(cos_t[:], th_r2[:], AF.Sin)

                # ---------- phase 1: projections (N=512 moving, d-granular) ----------
                with (
                    tc.tile_pool(name="w", bufs=1) as wpool,
                    tc.tile_pool(name="wv", bufs=3) as wvp,
                    tc.tile_pool(name="xt", bufs=1) as xtp,
                    tc.tile_pool(name="xin", bufs=3) as xin,
                    tc.tile_pool(name="rsc", bufs=1) as rsc,
                    tc.tile_pool(name="rout", bufs=2) as rout,
                    tc.tile_pool(name="ps_tr", bufs=2, space="PSUM") as ps_tr,
                    tc.tile_pool(name="ps_mm", bufs=1, space="PSUM") as ps_mm,
                ):
                    # per-d weight tiles: [0:512 wq | 512:768 wk]
                    w_d = []
                    for d in range(DC):
                        wt = wpool.tile([128, 768], f32r, tag=f"w{d}", name=f"w{d}")
                        rs = slice(d * 128, (d + 1) * 128)
                        nc.sync.dma_start(wt[:, 0:512], wq_in[rs, :].bitcast(f32r))
                        nc.sync.dma_start(wt[:, 512:768], wk_in[rs, :].bitcast(f32r))
                        w_d.append(wt)

                    def rope_pair(pA, pB, ts0, n):
                        cos_s = cos_t[:, ts0:ts0 + n]
                        sin_s = sin_t[:, ts0:ts0 + n]
                        t1 = rsc.tile([128, 512], f32, tag="t1", name="t1")
                        t2 = rsc.tile([128, 512], f32, tag="t2", name="t2")
                        rot0 = rout.tile([128, 512], f32, tag="rot0", name="rot0")
                        rot1 = rout.tile([128, 512], f32, tag="rot1", name="rot1")
                        nc.vector.tensor_tensor(t1[:], pA, cos_s, Alu.mult)
                        nc.vector.tensor_tensor(t2[:], pB, sin_s, Alu.mult)
                        nc.vector.tensor_tensor(rot0[:], t1[:], t2[:], Alu.subtract)
                        nc.vector.tensor_tensor(t1[:], pB, cos_s, Alu.mult)
                        nc.vector.tensor_tensor(t2[:], pA, sin_s, Alu.mult)
                        nc.vector.tensor_tensor(rot1[:], t1[:], t2[:], Alu.add)
                        return rot0, rot1

                    for tb in range(T // 512):          # 4 t-blocks of 512
                        ts0 = tb * 512
                        xT_d = []
                        for d in range(DC):
                            xt_t = xtp.tile([128, 512], f32r, tag=f"xT{d}",
                                            name=f"xT{d}")
                            xT_d.append(xt_t)

                        # pass 1: transposes + q (4 groups) + k (2 groups)
                        g_off = (0, 128, 256, 384, 512, 640)
                        ps_g = []
                        for g in range(6):
                            pg = ps_mm.tile([128, 512], f32, tag=f"g{g}",
                                            name=f"g{g}")
                            ps_g.append(pg)
                        for d in range(DC):
                            x_d = xin.tile([128, 512], f32r, tag="xd", name="xd")
                            for tc2 in range(4):
                                nc.sync.dma_start(
                                    x_d[:, tc2 * 128:(tc2 + 1) * 128],
                                    x_in[ts0 + tc2 * 128:ts0 + (tc2 + 1) * 128,
                                         d * 128:(d + 1) * 128].bitcast(f32r))
                            for tc2 in range(4):
                                tp = ps_tr.tile([128, 128], f32r, tag="tr",
                                                name="tr")
                                nc.tensor.transpose(
                                    tp[:], x_d[:, tc2 * 128:(tc2 + 1) * 128],
                                    ident_r[:])
                                nc.vector.tensor_copy(
                                    xT_d[d][:, tc2 * 128:(tc2 + 1) * 128], tp[:])
                            for g in range(6):
                                nc.tensor.matmul(
                                    ps_g[g][:],
                                    w_d[d][:, g_off[g]:g_off[g] + 128],
                                    xT_d[d][:],
                                    start=(d == 0), stop=(d == DC - 1))

                        for pair in range(3):           # q0, q1, k
                            rot0, rot1 = rope_pair(ps_g[2 * pair][:],
                                                   ps_g[2 * pair + 1][:], ts0, 512)
                            if pair < 2:
                                base = pair * 256
                                nc.sync.dma_start(
                                    qT_d[base:base + 128, ts0:ts0 + 512], rot0[:])
                                nc.sync.dma_start(
                                    qT_d[base + 128:base + 256, ts0:ts0 + 512],
                                    rot1[:])
                            else:
                                nc.sync.dma_start(kT_d[0:128, ts0:ts0 + 512],
                                                  rot0[:])
                                nc.sync.dma_start(kT_d[128:256, ts0:ts0 + 512],
                                                  rot1[:])

                        # pass 2: v (2 groups, reuse g0/g1 banks)
                        ps_v0 = ps_mm.tile([128, 512], f32, tag="g0", name="ps_v0")
                        ps_v1 = ps_mm.tile([128, 512], f32, tag="g1", name="ps_v1")
                        for d in range(DC):
                            wv_t = wvp.tile([128, 256], f32r, tag="wv", name="wv_t")
                            nc.sync.dma_start(
                                wv_t[:],
                                wv_in[d * 128:(d + 1) * 128, :].bitcast(f32r))
                            nc.tensor.matmul(ps_v0[:], wv_t[:, 0:128], xT_d[d][:],
                                             start=(d == 0), stop=(d == DC - 1))
                            nc.tensor.matmul(ps_v1[:], wv_t[:, 128:256], xT_d[d][:],
                                             start=(d == 0), stop=(d == DC - 1))
                        for c, ps_vc in enumerate((ps_v0, ps_v1)):
                            v_sb = rout.tile([128, 512], f32, tag="v_sb",
                                             name="v_sb")
                            nc.vector.tensor_copy(v_sb[:], ps_vc[:])
                            nc.sync.dma_start(vT_d[c * 128:(c + 1) * 128,
                                                   ts0:ts0 + 512], v_sb[:])

            # ---------- phase 2: attention (head-outer, split AllToAll) ----------
            with (
                tc.tile_pool(name="kv", bufs=1) as kv,
                tc.tile_pool(name="msk", bufs=1) as msk,
                tc.tile_pool(name="qp", bufs=2) as qp,
                tc.tile_pool(name="pp", bufs=4) as pp,
                tc.tile_pool(name="enc", bufs=2) as encp,
                tc.tile_pool(name="ps_l", bufs=3, space="PSUM") as ps_lp,
                tc.tile_pool(name="ps_e", bufs=1, space="PSUM") as ps_ep,
                tc.tile_pool(name="ps_b", bufs=1, space="PSUM") as ps_bp,
            ):
                # masks (additive, applied pre-softcap)
                mask_tiles = {}
                for dd in CAUSAL_DD:
                    m = msk.tile([128, 512], f32, tag=f"mc{dd}")
                    nc.gpsimd.memset(m[:], 0.0)
                    # live iff i - j + dd <= 0  ⟺  j - i - dd >= 0
                    nc.gpsimd.affine_select(
                        out=m[:], in_=m[:], compare_op=Alu.is_ge, fill=MASK_VAL,
                        base=-dd, pattern=[[1, 512]], channel_multiplier=-1)
                    mask_tiles[dd] = m
                for dd in WINDOW_DD:
                    m = msk.tile([128, 512], f32, tag=f"mw{dd}")
                    nc.gpsimd.memset(m[:], 0.0)
                    nc.gpsimd.affine_select(
                        out=m[:], in_=m[:], compare_op=Alu.is_gt, fill=MASK_VAL,
                        base=dd + WINDOW, pattern=[[-1, 512]], channel_multiplier=1)
                    mask_tiles[dd] = m

                kT_c = []
                for c in range(2):
                    kt = kv.tile([128, T], f32r, tag=f"kt{c}")
                    nc.sync.dma_start(kt[:],
                                      kT_d[c * 128:(c + 1) * 128, :].bitcast(f32r))
                    kT_c.append(kt)

                # vT [256, T] -> v_all [S-part, H-free] via 32 PE transposes
                v_all = kv.tile([128, 16 * 256], f32r, tag="v_all")
                for c in range(2):
                    vt_sb = kv.tile([128, T], f32r, tag=f"vt{c}")
                    nc.sync.dma_start(vt_sb[:],
                                      vT_d[c * 128:(c + 1) * 128, :].bitcast(f32r))
                    for sj in range(16):
                        tp = ps_bp.tile([128, 128], f32r, tag="vtr")
                        nc.tensor.transpose(
                            tp[:], vt_sb[:, sj * 128:(sj + 1) * 128], ident_r[:])
                        nc.vector.tensor_copy(
                            v_all[:, sj * 256 + c * 128:sj * 256 + c * 128 + 128],
                            tp[:])

                for lh in range(2):
                    for tb in range(T // 512):
                        js = _live_chunks(tb)
                        q_c = []
                        for c in range(2):
                            qt = qp.tile([128, 512], f32r, tag=f"q{c}")
                            nc.sync.dma_start(
                                qt[:],
                                qT_d[lh * 256 + c * 128:lh * 256 + (c + 1) * 128,
                                     tb * 512:(tb + 1) * 512].bitcast(f32r))
                            q_c.append(qt)
                        e0 = ps_ep.tile([128, 512], f32, tag="e0")
                        e1 = ps_ep.tile([128, 512], f32, tag="e1")
                        den = ps_ep.tile([1, 512], f32, tag="den")
                        for idx, sj in enumerate(js):
                            ps_l = ps_lp.tile([128, 512], f32, tag="l")
                            nc.tensor.matmul(ps_l[:],
                                             kT_c[0][:, sj * 128:(sj + 1) * 128],
                                             q_c[0][:], start=True, stop=False)
                            nc.tensor.matmul(ps_l[:],
                                             kT_c[1][:, sj * 128:(sj + 1) * 128],
                                             q_c[1][:], start=False, stop=True)
                            dd = sj * 128 - tb * 512
                            if dd in mask_tiles:
                                nc.vector.tensor_tensor(ps_l[:], ps_l[:],
                                                        mask_tiles[dd][:], Alu.add)
                            tmp = pp.tile([128, 512], f32, tag="tmp")
                            nc.scalar.activation(tmp[:], ps_l[:], AF.Tanh,
                                                 scale=TANH_SCALE)
                            pj = pp.tile([128, 512], f32r, tag="pj")
                            nc.scalar.activation(pj[:], tmp[:], AF.Exp,
                                                 scale=SOFT_CAP)
                            first, last = idx == 0, idx == len(js) - 1
                            nc.tensor.matmul(e0[:],
                                             v_all[:, sj * 256:sj * 256 + 128],
                                             pj[:], start=first, stop=last)
                            nc.tensor.matmul(e1[:],
                                             v_all[:, sj * 256 + 128:sj * 256 + 256],
                                             pj[:], start=first, stop=last)
                            nc.tensor.matmul(den[:], ones_col_r[:], pj[:],
                                             start=first, stop=last)
                        recip = encp.tile([1, 512], f32, tag="recip")
                        nc.vector.reciprocal(recip[:], den[:])
                        bc = ps_bp.tile([128, 512], f32, tag="bc")
                        nc.tensor.matmul(bc[:], ones_row_f[:], recip[:],
                                         start=True, stop=True)
                        bc_sb = encp.tile([128, 512], f32, tag="bc_sb")
                        nc.scalar.copy(bc_sb[:], bc[:])
                        for c, e_ps in enumerate((e0, e1)):
                            e_sb = encp.tile([128, 512], f32, tag=f"e_sb{c}")
                            nc.vector.tensor_tensor(e_sb[:], e_ps[:], bc_sb[:],
                                                    Alu.mult)
                            r0 = c * 128
                            nc.sync.dma_start(cc_in[lh][tb * 2, r0:r0 + 128, :],
                                              e_sb[:, 0:256])
                            nc.sync.dma_start(cc_in[lh][tb * 2 + 1, r0:r0 + 128, :],
                                              e_sb[:, 256:512])

                    nc.gpsimd.collective_compute(
                        "AllToAll", Alu.bypass,
                        replica_groups=[list(range(N_CORES))],
                        ins=[cc_in[lh][:]], outs=[cc_out[lh][:]])

            # ---------- phase 3: output projection for the local T-slice ----------
            with (
                tc.tile_pool(name="ge", bufs=1) as ge,
                tc.tile_pool(name="wo", bufs=2) as wop,
                tc.tile_pool(name="o", bufs=2) as op_,
                tc.tile_pool(name="ps_o", bufs=2, space="PSUM") as ps_op,
            ):
                # accumulation ordered lh-first so phase 3 starts after A2A#0;
                # chunk i covers global rows [(src*4+lh*2+ph)*128 ...)
                ORDER = [(lh, src, ph) for lh in range(2) for src in range(8)
                         for ph in range(2)]
                enc_c = []
                for i, (lh, src, ph) in enumerate(ORDER):
                    ec = ge.tile([128, 256], f32r, tag=f"e{i}", name=f"e{i}")
                    nc.sync.dma_start(
                        ec[:],
                        cc_out[lh][src, ph * 128:(ph + 1) * 128, :].bitcast(f32r))
                    enc_c.append(ec)
                for db in range(D // 512):
                    wo_k = []
                    for i, (lh, src, ph) in enumerate(ORDER):
                        grow = (src * 4 + lh * 2 + ph) * 128
                        wt = wop.tile([128, 512], f32r, tag=f"wo{i}",
                                      name=f"wo{i}")
                        nc.sync.dma_start(
                            wt[:],
                            wo_in[grow:grow + 128,
                                  db * 512:(db + 1) * 512].bitcast(f32r))
                        wo_k.append(wt)
                    for tc2 in range(2):
                        ps_o = ps_op.tile([128, 512], f32, tag="o", name="ps_o")
                        for i in range(32):
                            nc.tensor.matmul(
                                ps_o[:],
                                enc_c[i][:, tc2 * 128:(tc2 + 1) * 128],
                                wo_k[i][:],
                                start=(i == 0), stop=(i == 31))
                        o_sb = op_.tile([128, 512], f32, tag="o_sb", name="o_sb")
                        nc.vector.tensor_copy(o_sb[:], ps_o[:])
                        nc.sync.dma_start(
                            out_ext[tc2 * 128:(tc2 + 1) * 128,
                                    db * 512:(db + 1) * 512], o_sb[:])

    nc.compile()
    return nc


_CACHE = {}
LAST_RESULTS = None


def _get_module():
    if "nc" not in _CACHE:
        _CACHE["nc"] = _build_module()
    return _CACHE["nc"]


def kernel(x, segment_pos, attn_mask, wq, wkv, wo):
    global LAST_RESULTS
    x = np.asarray(x, dtype=np.float32)
    segment_pos = np.asarray(segment_pos, dtype=np.int32)
    wq = np.asarray(wq, dtype=np.float32)
    wkv = np.asarray(wkv, dtype=np.float32)
    wo = np.asarray(wo, dtype=np.float32)

    nc = _get_module()

    consts = np.zeros((128, 130), dtype=np.float32)
    consts[:, 0:128] = np.eye(128, dtype=np.float32)
    consts[:, 128] = 1.0
    consts[:, 129] = (10000.0 ** (-np.arange(128) / 128.0)).astype(np.float32)

    x2d = np.ascontiguousarray(x[0])
    pos = np.ascontiguousarray(segment_pos[0:1])
    wo_flat = np.ascontiguousarray(wo.reshape(4096, D))

    in_maps = []
    for i in range(N_CORES):
        in_maps.append({
            "x": x2d,
            "pos": pos,
            "wq": np.ascontiguousarray(
                np.concatenate([wq[2 * i], wq[2 * i + 1]], axis=1)),
            "wk": np.ascontiguousarray(wkv[0, i]),
            "wv": np.ascontiguousarray(wkv[1, i]),
            "wo": wo_flat,
            "consts": consts,
        })

    LAST_RESULTS = run_bass_kernel_spmd(nc, in_maps,
                                        core_ids=list(range(N_CORES)))
    out = np.concatenate([LAST_RESULTS.results[i]["out"]
                          for i in range(N_CORES)], axis=0)
    return out[None, :, :].astype(np.float32)


# revision 16
# speedup vs baseline: 1.1110x; 1.1110x over previous
"""Trainium2 Bass kernel for sliding-window GQA attention (nn_Attention_12610023981270).

Sharding: 8 cores, head-parallel — core i owns q-heads {2i, 2i+1} and kv-head i
for projections + attention, then an AllToAll switches to sequence-parallel for
the output projection (core i produces output rows [256*i, 256*(i+1))).

Everything on-chip stays "transposed" ([feature, token]) so the only transposes
needed are x itself (PE transpose-mode), and matmuls run in float32r
(full-rate ~1.6e-4 rel-err fp32 mode of the PE).

Model: B=1, T=2048, D=3584, 16 q-heads / 8 kv-heads, head_dim 256,
RoPE, query_scale 1/16, logit softcap 50, causal + sliding window 1024.
"""
import sys

if '/opt/trn_rl_repo' not in sys.path:
    sys.path.insert(0, '/opt/trn_rl_repo')

import numpy as np

import concourse.bass as bass
import concourse.mybir as mybir
import concourse.tile as tile
from concourse import bacc
from concourse.bass_utils import run_bass_kernel_spmd

f32 = mybir.dt.float32
f32r = mybir.dt.float32r
i32 = mybir.dt.int32
AF = mybir.ActivationFunctionType
Alu = mybir.AluOpType

N_CORES = 8
T, D, HD = 2048, 3584, 256
DC = D // 128            # 28 d-chunks
TWO_PI = 6.283185307179586
HALF_PI = 1.5707963267948966
SOFT_CAP = 50.0
QUERY_SCALAR = 0.0625
WINDOW = 1024
MASK_VAL = -1.0e6
TANH_SCALE = QUERY_SCALAR / SOFT_CAP   # folds query scaling into the softcap

# per-tb512 live s-chunks and the additive-mask pattern offsets
CAUSAL_DD = (0, 128, 256, 384)
WINDOW_DD = (-1024, -896, -768, -640)


def _live_chunks(tb):
    t0 = tb * 512
    smin = max(0, t0 - (WINDOW - 1))
    smax = t0 + 511
    return list(range(smin // 128, smax // 128 + 1))


def _build_module():
    nc = bacc.Bacc("TRN2", target_bir_lowering=False, debug=False,
                   num_devices=N_CORES)

    x_in = nc.declare_dram_parameter("x", [T, D], f32, isOutput=False)
    pos_in = nc.declare_dram_parameter("pos", [1, T], i32, isOutput=False)
    wq_in = nc.declare_dram_parameter("wq", [D, 512], f32, isOutput=False)
    wk_in = nc.declare_dram_parameter("wk", [D, 256], f32, isOutput=False)
    wv_in = nc.declare_dram_parameter("wv", [D, 256], f32, isOutput=False)
    wo_in = nc.declare_dram_parameter("wo", [4096, D], f32, isOutput=False)
    # consts: [:, 0:128] identity, [:, 128] ones, [:, 129] inv_timescale
    consts_in = nc.declare_dram_parameter("consts", [128, 130], f32, isOutput=False)
    out_ext = nc.declare_dram_parameter("out", [T // N_CORES, D], f32, isOutput=True)

    qT_d = nc.dram_tensor("qT_d", [512, T], f32)
    kT_d = nc.dram_tensor("kT_d", [256, T], f32)
    v_d = nc.dram_tensor("v_d", [T, 256], f32)
    cc_in = nc.dram_tensor("cc_in", [8, 512, 256], f32)
    cc_out = nc.dram_tensor("cc_out", [8, 512, 256], f32)

    with tile.TileContext(nc) as tc:
        with tc.tile_pool(name="prep", bufs=1) as prep:
            ident_r = prep.tile([128, 128], f32r)
            nc.sync.dma_start(ident_r[:], consts_in[:, 0:128].bitcast(f32r))
            ones_col_r = prep.tile([128, 1], f32r)
            nc.sync.dma_start(ones_col_r[:], consts_in[:, 128:129].bitcast(f32r))
            ones_row_f = prep.tile([1, 128], f32)
            nc.sync.dma_start(ones_row_f[:],
                              consts_in[:, 128:129].rearrange("p one -> one p"))
            inv_ts = prep.tile([128, 1], f32)
            nc.sync.dma_start(inv_ts[:], consts_in[:, 129:130])

            # ---------- phase 0: RoPE sin/cos tables [128, T] ----------
            with tc.tile_pool(name="tables", bufs=1) as tbl:
                sin_t = tbl.tile([128, T], f32)
                cos_t = tbl.tile([128, T], f32)
                with (
                    tc.tile_pool(name="p0", bufs=1) as p0,
                    tc.tile_pool(name="ps0", bufs=2, space="PSUM") as ps0,
                ):
                    pos_i = p0.tile([1, T], i32)
                    nc.sync.dma_start(pos_i[:], pos_in[:])
                    pos_f = p0.tile([1, T], f32)
                    nc.vector.tensor_copy(pos_f[:], pos_i[:])
                    theta = p0.tile([128, T], f32)
                    for b in range(T // 512):
                        ps = ps0.tile([128, 512], f32, tag="bc0")
                        nc.tensor.matmul(ps[:], ones_row_f[:],
                                         pos_f[:, b * 512:(b + 1) * 512],
                                         start=True, stop=True)
                        nc.vector.tensor_scalar(theta[:, b * 512:(b + 1) * 512],
                                                ps[:], inv_ts[:], None, Alu.mult)

                    def range_reduce(dst, pre_add):
                        u = p0.tile([128, T], f32, tag="rr_u")
                        nc.vector.tensor_scalar(u[:], theta[:], pre_add,
                                                1.0 / TWO_PI, Alu.add, Alu.mult)
                        k_i = p0.tile([128, T], i32, tag="rr_k")
                        nc.vector.tensor_copy(k_i[:], u[:])
                        k_f = p0.tile([128, T], f32, tag="rr_kf")
                        nc.vector.tensor_copy(k_f[:], k_i[:])
                        r = p0.tile([128, T], f32, tag="rr_r")
                        nc.vector.tensor_tensor(r[:], u[:], k_f[:], Alu.subtract)
                        nc.vector.tensor_scalar(dst[:], r[:], TWO_PI, None,
                                                Alu.mult)

                    th_r = p0.tile([128, T], f32, tag="th_r")
                    range_reduce(th_r, 0.0)
                    nc.scalar.activation(sin_t[:], th_r[:], AF.Sin)
                    th_r2 = p0.tile([128, T], f32, tag="th_r")
                    range_reduce(th_r2, HALF_PI)
                    nc.scalar.activation(cos_t[:], th_r2[:], AF.Sin)

                # ---------- phase 1: projections ----------
                with (
                    tc.tile_pool(name="w", bufs=1) as wpool,
                    tc.tile_pool(name="xt", bufs=1) as xtp,
                    tc.tile_pool(name="xin", bufs=2) as xin,
                    tc.tile_pool(name="rope", bufs=4) as rope,
                    tc.tile_pool(name="ps_tr", bufs=2, space="PSUM") as ps_tr,
                    tc.tile_pool(name="ps_mm", bufs=1, space="PSUM") as ps_mm,
                ):
                    self_indent_marker = None  # noqa: F841
                # weights resident: w_all[:, d*1024 + {0:512 wq | 512:768 wk | 768:1024 wv}]
                w_all = wpool.tile([128, DC * 1024], f32r)
                for d in range(DC):
                    rs = slice(d * 128, (d + 1) * 128)
                    nc.sync.dma_start(w_all[:, d * 1024:d * 1024 + 512],
                                      wq_in[rs, :].bitcast(f32r))
                    nc.sync.dma_start(w_all[:, d * 1024 + 512:d * 1024 + 768],
                                      wk_in[rs, :].bitcast(f32r))
                    nc.sync.dma_start(w_all[:, d * 1024 + 768:d * 1024 + 1024],
                                      wv_in[rs, :].bitcast(f32r))

                for tb in range(T // 256):          # 8 t-blocks of 256
                    ts0 = tb * 256
                    # transpose x[t-block] -> xT_buf [128 D, 256 T] per d-chunk
                    xT_buf = xtp.tile([128, DC * 256], f32r, tag="xT")
                    for half in range(2):           # x tile halves [128, 1792]
                        for tc2 in range(2):        # two 128-row t-chunks
                            x_t = xin.tile([128, DC // 2 * 128], f32r,
                                           tag=f"x{tc2}")
                            nc.sync.dma_start(
                                x_t[:],
                                x_in[ts0 + tc2 * 128:ts0 + (tc2 + 1) * 128,
                                     half * 1792:(half + 1) * 1792].bitcast(f32r))
                            for dd_ in range(DC // 2):
                                d = half * (DC // 2) + dd_
                                tp = ps_tr.tile([128, 128], f32r, tag="tr")
                                nc.tensor.transpose(
                                    tp[:], x_t[:, dd_ * 128:(dd_ + 1) * 128],
                                    ident_r[:])
                                nc.vector.tensor_copy(
                                    xT_buf[:, d * 256 + tc2 * 128:
                                           d * 256 + tc2 * 128 + 128], tp[:])

                    # v projection: out [T-part, H-free] — lhsT = xT chunk [D, T128],
                    # rhs = wv [D, 256]; two t-chunks packed 2-up in one bank
                    ps_v = ps_mm.tile([128, 512], f32, tag="psv")
                    for tc2 in range(2):
                        for d in range(DC):
                            nc.tensor.matmul(
                                ps_v[:, tc2 * 256:(tc2 + 1) * 256],
                                xT_buf[:, d * 256 + tc2 * 128:
                                       d * 256 + tc2 * 128 + 128],
                                w_all[:, d * 1024 + 768:d * 1024 + 1024],
                                start=(d == 0), stop=(d == DC - 1))

                    for tc2 in range(2):
                        v_sb = rope.tile([128, 256], f32, tag="v_sb")
                        nc.vector.tensor_copy(v_sb[:],
                                              ps_v[:, tc2 * 256:(tc2 + 1) * 256])
                        nc.sync.dma_start(
                            v_d[ts0 + tc2 * 128:ts0 + (tc2 + 1) * 128, :], v_sb[:])

                    # qT / kT projections + RoPE
                    cos_s = cos_t[:, ts0:ts0 + 256]
                    sin_s = sin_t[:, ts0:ts0 + 256]
                    for pi_, w_off in enumerate((0, 256, 512)):  # q0, q1, k
                        ps_qk = ps_mm.tile([128, 512], f32, tag=f"qk{pi_}")
                        for c in range(2):
                            for d in range(DC):
                                nc.tensor.matmul(
                                    ps_qk[:, c * 256:(c + 1) * 256],
                                    w_all[:, d * 1024 + w_off + c * 128:
                                          d * 1024 + w_off + c * 128 + 128],
                                    xT_buf[:, d * 256:(d + 1) * 256],
                                    start=(d == 0), stop=(d == DC - 1))
                        pA = ps_qk[:, 0:256]
                        pB = ps_qk[:, 256:512]
                        t1 = rope.tile([128, 256], f32, tag="t1")
                        t2 = rope.tile([128, 256], f32, tag="t2")
                        rot0 = rope.tile([128, 256], f32, tag="rot0")
                        rot1 = rope.tile([128, 256], f32, tag="rot1")
                        nc.vector.tensor_tensor(t1[:], pA, cos_s, Alu.mult)
                        nc.vector.tensor_tensor(t2[:], pB, sin_s, Alu.mult)
                        nc.vector.tensor_tensor(rot0[:], t1[:], t2[:], Alu.subtract)
                        nc.vector.tensor_tensor(t1[:], pB, cos_s, Alu.mult)
                        nc.vector.tensor_tensor(t2[:], pA, sin_s, Alu.mult)
                        nc.vector.tensor_tensor(rot1[:], t1[:], t2[:], Alu.add)
                        if pi_ < 2:  # q heads
                            base = pi_ * 256
                            nc.sync.dma_start(qT_d[base:base + 128, ts0:ts0 + 256],
                                              rot0[:])
                            nc.sync.dma_start(qT_d[base + 128:base + 256,
                                                   ts0:ts0 + 256], rot1[:])
                        else:        # k
                            nc.sync.dma_start(kT_d[0:128, ts0:ts0 + 256], rot0[:])
                            nc.sync.dma_start(kT_d[128:256, ts0:ts0 + 256], rot1[:])

            # ---------- phase 2: attention ----------
            with (
                tc.tile_pool(name="kv", bufs=1) as kv,
                tc.tile_pool(name="msk", bufs=1) as msk,
                tc.tile_pool(name="qp", bufs=2) as qp,
                tc.tile_pool(name="pp", bufs=4) as pp,
                tc.tile_pool(name="enc", bufs=2) as encp,
                tc.tile_pool(name="ps_l", bufs=3, space="PSUM") as ps_lp,
                tc.tile_pool(name="ps_e", bufs=1, space="PSUM") as ps_ep,
                tc.tile_pool(name="ps_b", bufs=1, space="PSUM") as ps_bp,
            ):
                # masks (additive, applied pre-softcap)
                mask_tiles = {}
                for dd in CAUSAL_DD:
                    m = msk.tile([128, 512], f32, tag=f"mc{dd}")
                    nc.gpsimd.memset(m[:], 0.0)
                    # live iff i - j + dd <= 0  ⟺  j - i - dd >= 0
                    nc.gpsimd.affine_select(
                        out=m[:], in_=m[:], compare_op=Alu.is_ge, fill=MASK_VAL,
                        base=-dd, pattern=[[1, 512]], channel_multiplier=-1)
                    mask_tiles[dd] = m
                for dd in WINDOW_DD:
                    m = msk.tile([128, 512], f32, tag=f"mw{dd}")
                    nc.gpsimd.memset(m[:], 0.0)
                    nc.gpsimd.affine_select(
                        out=m[:], in_=m[:], compare_op=Alu.is_gt, fill=MASK_VAL,
                        base=dd + WINDOW, pattern=[[-1, 512]], channel_multiplier=1)
                    mask_tiles[dd] = m

                kT_c = []
                for c in range(2):
                    kt = kv.tile([128, T], f32r, tag=f"kt{c}")
                    nc.sync.dma_start(kt[:], kT_d[c * 128:(c + 1) * 128, :].bitcast(f32r))
                    kT_c.append(kt)
                v_all = kv.tile([128, 16 * 256], f32r, tag="v_all")
                for sj in range(16):
                    nc.sync.dma_start(v_all[:, sj * 256:(sj + 1) * 256],
                                      v_d[sj * 128:(sj + 1) * 128, :].bitcast(f32r))

                for tb in range(T // 512):
                    js = _live_chunks(tb)
                    for lh in range(2):
                        q_c = []
                        for c in range(2):
                            qt = qp.tile([128, 512], f32r, tag=f"q{c}")
                            nc.sync.dma_start(
                                qt[:],
                                qT_d[lh * 256 + c * 128:lh * 256 + (c + 1) * 128,
                                     tb * 512:(tb + 1) * 512].bitcast(f32r))
                            q_c.append(qt)
                        e0 = ps_ep.tile([128, 512], f32, tag="e0")
                        e1 = ps_ep.tile([128, 512], f32, tag="e1")
                        den = ps_ep.tile([1, 512], f32, tag="den")
                        for idx, sj in enumerate(js):
                            ps_l = ps_lp.tile([128, 512], f32, tag="l")
                            nc.tensor.matmul(ps_l[:], kT_c[0][:, sj * 128:(sj + 1) * 128],
                                             q_c[0][:], start=True, stop=False)
                            nc.tensor.matmul(ps_l[:], kT_c[1][:, sj * 128:(sj + 1) * 128],
                                             q_c[1][:], start=False, stop=True)
                            dd = sj * 128 - tb * 512
                            if dd in mask_tiles:
                                nc.vector.tensor_tensor(ps_l[:], ps_l[:],
                                                        mask_tiles[dd][:], Alu.add)
                            tmp = pp.tile([128, 512], f32, tag="tmp")
                            nc.scalar.activation(tmp[:], ps_l[:], AF.Tanh,
                                                 scale=TANH_SCALE)
                            pj = pp.tile([128, 512], f32r, tag="pj")
                            nc.scalar.activation(pj[:], tmp[:], AF.Exp,
                                                 scale=SOFT_CAP)
                            first, last = idx == 0, idx == len(js) - 1
                            nc.tensor.matmul(e0[:], v_all[:, sj * 256:sj * 256 + 128],
                                             pj[:], start=first, stop=last)
                            nc.tensor.matmul(e1[:], v_all[:, sj * 256 + 128:sj * 256 + 256],
                                             pj[:], start=first, stop=last)
                            nc.tensor.matmul(den[:], ones_col_r[:], pj[:],
                                             start=first, stop=last)
                        recip = encp.tile([1, 512], f32, tag="recip")
                        nc.vector.reciprocal(recip[:], den[:])
                        bc = ps_bp.tile([128, 512], f32, tag="bc")
                        nc.tensor.matmul(bc[:], ones_row_f[:], recip[:],
                                         start=True, stop=True)
                        bc_sb = encp.tile([128, 512], f32, tag="bc_sb")
                        nc.scalar.copy(bc_sb[:], bc[:])
                        for c, e_ps in enumerate((e0, e1)):
                            e_sb = encp.tile([128, 512], f32, tag=f"e_sb{c}")
                            nc.vector.tensor_tensor(e_sb[:], e_ps[:], bc_sb[:],
                                                    Alu.mult)
                            r0 = lh * 256 + c * 128
                            nc.sync.dma_start(cc_in[tb * 2, r0:r0 + 128, :],
                                              e_sb[:, 0:256])
                            nc.sync.dma_start(cc_in[tb * 2 + 1, r0:r0 + 128, :],
                                              e_sb[:, 256:512])

            nc.gpsimd.collective_compute(
                "AllToAll", Alu.bypass,
                replica_groups=[list(range(N_CORES))],
                ins=[cc_in[:]], outs=[cc_out[:]])

            # ---------- phase 3: output projection for the local T-slice ----------
            with (
                tc.tile_pool(name="ge", bufs=1) as ge,
                tc.tile_pool(name="wo", bufs=2) as wop,
                tc.tile_pool(name="o", bufs=2) as op_,
                tc.tile_pool(name="ps_o", bufs=2, space="PSUM") as ps_op,
            ):
                enc_all = ge.tile([128, 32 * 256], f32r)
                flat = cc_out[:].rearrange("c p f -> (c p) f")
                for k in range(32):
                    nc.sync.dma_start(enc_all[:, k * 256:(k + 1) * 256],
                                      flat[k * 128:(k + 1) * 128, :].bitcast(f32r))
                for db in range(D // 512):
                    wo_buf = wop.tile([128, 32 * 512], f32r, tag="wo")
                    for k in range(32):
                        nc.sync.dma_start(
                            wo_buf[:, k * 512:(k + 1) * 512],
                            wo_in[k * 128:(k + 1) * 128,
                                  db * 512:(db + 1) * 512].bitcast(f32r))
                    for tc2 in range(2):
                        ps_o = ps_op.tile([128, 512], f32, tag="o")
                        for k in range(32):
                            nc.tensor.matmul(
                                ps_o[:],
                                enc_all[:, k * 256 + tc2 * 128:k * 256 + tc2 * 128 + 128],
                                wo_buf[:, k * 512:(k + 1) * 512],
                                start=(k == 0), stop=(k == 31))
                        o_sb = op_.tile([128, 512], f32, tag="o_sb")
                        nc.vector.tensor_copy(o_sb[:], ps_o[:])
                        nc.sync.dma_start(
                            out_ext[tc2 * 128:(tc2 + 1) * 128,
                                    db * 512:(db + 1) * 512], o_sb[:])

    nc.compile()
    return nc


_CACHE = {}
LAST_RESULTS = None


def _get_module():
    if "nc" not in _CACHE:
        _CACHE["nc"] = _build_module()
    return _CACHE["nc"]


def kernel(x, segment_pos, attn_mask, wq, wkv, wo):
    global LAST_RESULTS
    x = np.asarray(x, dtype=np.float32)
    segment_pos = np.asarray(segment_pos, dtype=np.int32)
    wq = np.asarray(wq, dtype=np.float32)
    wkv = np.asarray(wkv, dtype=np.float32)
    wo = np.asarray(wo, dtype=np.float32)

    nc = _get_module()

    consts = np.zeros((128, 130), dtype=np.float32)
    consts[:, 0:128] = np.eye(128, dtype=np.float32)
    consts[:, 128] = 1.0
    consts[:, 129] = (10000.0 ** (-np.arange(128) / 128.0)).astype(np.float32)

    x2d = np.ascontiguousarray(x[0])
    pos = np.ascontiguousarray(segment_pos[0:1])
    wo_flat = np.ascontiguousarray(wo.reshape(4096, D))

    in_maps = []
    for i in range(N_CORES):
        in_maps.append({
            "x": x2d,
            "pos": pos,
            "wq": np.ascontiguousarray(
                np.concatenate([wq[2 * i], wq[2 * i + 1]], axis=1)),
            "wk": np.ascontiguousarray(wkv[0, i]),
            "wv": np.ascontiguousarray(wkv[1, i]),
            "wo": wo_flat,
            "consts": consts,
        })

    LAST_RESULTS = run_bass_kernel_spmd(nc, in_maps,
                                        core_ids=list(range(N_CORES)))
    out = np.concatenate([LAST_RESULTS.results[i]["out"]
                          for i in range(N_CORES)], axis=0)
    return out[None, :, :].astype(np.float32)


# revision 18
# speedup vs baseline: 1.1587x; 1.0429x over previous
"""Trainium2 Bass kernel for sliding-window GQA attention (nn_Attention_12610023981270).

Sharding: 8 cores, head-parallel — core i owns q-heads {2i, 2i+1} and kv-head i
for projections + attention, then an AllToAll switches to sequence-parallel for
the output projection (core i produces output rows [256*i, 256*(i+1))).

Everything on-chip stays "transposed" ([feature, token]) so the only transposes
needed are x itself (PE transpose-mode), and matmuls run in float32r
(full-rate ~1.6e-4 rel-err fp32 mode of the PE).

Model: B=1, T=2048, D=3584, 16 q-heads / 8 kv-heads, head_dim 256,
RoPE, query_scale 1/16, logit softcap 50, causal + sliding window 1024.
"""
import sys

if '/opt/trn_rl_repo' not in sys.path:
    sys.path.insert(0, '/opt/trn_rl_repo')

import numpy as np

import concourse.bass as bass
import concourse.mybir as mybir
import concourse.tile as tile
from concourse import bacc
from concourse.bass_utils import run_bass_kernel_spmd

f32 = mybir.dt.float32
f32r = mybir.dt.float32r
i32 = mybir.dt.int32
AF = mybir.ActivationFunctionType
Alu = mybir.AluOpType

N_CORES = 8
T, D, HD = 2048, 3584, 256
DC = D // 128            # 28 d-chunks
TWO_PI = 6.283185307179586
HALF_PI = 1.5707963267948966
SOFT_CAP = 50.0
QUERY_SCALAR = 0.0625
WINDOW = 1024
MASK_VAL = -1.0e6
TANH_SCALE = QUERY_SCALAR / SOFT_CAP   # folds query scaling into the softcap

# per-tb512 live s-chunks and the additive-mask pattern offsets
CAUSAL_DD = (0, 128, 256, 384)
WINDOW_DD = (-1024, -896, -768, -640)


def _live_chunks(tb):
    t0 = tb * 512
    smin = max(0, t0 - (WINDOW - 1))
    smax = t0 + 511
    return list(range(smin // 128, smax // 128 + 1))


def _build_module():
    nc = bacc.Bacc("TRN2", target_bir_lowering=False, debug=False,
                   num_devices=N_CORES)

    x_in = nc.declare_dram_parameter("x", [T, D], f32, isOutput=False)
    pos_in = nc.declare_dram_parameter("pos", [1, T], i32, isOutput=False)
    wq_in = nc.declare_dram_parameter("wq", [D, 512], f32, isOutput=False)
    wk_in = nc.declare_dram_parameter("wk", [D, 256], f32, isOutput=False)
    wv_in = nc.declare_dram_parameter("wv", [D, 256], f32, isOutput=False)
    wo_in = nc.declare_dram_parameter("wo", [4096, D], f32, isOutput=False)
    # consts: [:, 0:128] identity, [:, 128] ones, [:, 129] inv_timescale
    consts_in = nc.declare_dram_parameter("consts", [128, 130], f32, isOutput=False)
    out_ext = nc.declare_dram_parameter("out", [T // N_CORES, D], f32, isOutput=True)

    qT_d = nc.dram_tensor("qT_d", [512, T], f32)
    kT_d = nc.dram_tensor("kT_d", [256, T], f32)
    v_d = nc.dram_tensor("v_d", [T, 256], f32)
    cc_in = [nc.dram_tensor(f"cc_in{h}", [8, 256, 256], f32) for h in range(2)]
    cc_out = [nc.dram_tensor(f"cc_out{h}", [8, 256, 256], f32) for h in range(2)]

    with tile.TileContext(nc) as tc:
        with tc.tile_pool(name="prep", bufs=1) as prep:
            ident_r = prep.tile([128, 128], f32r)
            nc.sync.dma_start(ident_r[:], consts_in[:, 0:128].bitcast(f32r))
            ones_col_r = prep.tile([128, 1], f32r)
            nc.sync.dma_start(ones_col_r[:], consts_in[:, 128:129].bitcast(f32r))
            ones_row_f = prep.tile([1, 128], f32)
            nc.sync.dma_start(ones_row_f[:],
                              consts_in[:, 128:129].rearrange("p one -> one p"))
            inv_ts = prep.tile([128, 1], f32)
            nc.sync.dma_start(inv_ts[:], consts_in[:, 129:130])

            # ---------- phase 0: RoPE sin/cos tables [128, T] ----------
            with tc.tile_pool(name="tables", bufs=1) as tbl:
                sin_t = tbl.tile([128, T], f32)
                cos_t = tbl.tile([128, T], f32)
                with (
                    tc.tile_pool(name="p0", bufs=1) as p0,
                    tc.tile_pool(name="ps0", bufs=2, space="PSUM") as ps0,
                ):
                    pos_i = p0.tile([1, T], i32)
                    nc.sync.dma_start(pos_i[:], pos_in[:])
                    pos_f = p0.tile([1, T], f32)
                    nc.vector.tensor_copy(pos_f[:], pos_i[:])
                    theta = p0.tile([128, T], f32)
                    for b in range(T // 512):
                        ps = ps0.tile([128, 512], f32, tag="bc0")
                        nc.tensor.matmul(ps[:], ones_row_f[:],
                                         pos_f[:, b * 512:(b + 1) * 512],
                                         start=True, stop=True)
                        nc.vector.tensor_scalar(theta[:, b * 512:(b + 1) * 512],
                                                ps[:], inv_ts[:], None, Alu.mult)

                    def range_reduce(dst, pre_add):
                        u = p0.tile([128, T], f32, tag="rr_u")
                        nc.vector.tensor_scalar(u[:], theta[:], pre_add,
                                                1.0 / TWO_PI, Alu.add, Alu.mult)
                        k_i = p0.tile([128, T], i32, tag="rr_k")
                        nc.vector.tensor_copy(k_i[:], u[:])
                        k_f = p0.tile([128, T], f32, tag="rr_kf")
                        nc.vector.tensor_copy(k_f[:], k_i[:])
                        r = p0.tile([128, T], f32, tag="rr_r")
                        nc.vector.tensor_tensor(r[:], u[:], k_f[:], Alu.subtract)
                        nc.vector.tensor_scalar(dst[:], r[:], TWO_PI, None,
                                                Alu.mult)

                    th_r = p0.tile([128, T], f32, tag="th_r")
                    range_reduce(th_r, 0.0)
                    nc.scalar.activation(sin_t[:], th_r[:], AF.Sin)
                    th_r2 = p0.tile([128, T], f32, tag="th_r")
                    range_reduce(th_r2, HALF_PI)
                    nc.scalar.activation(cos_t[:], th_r2[:], AF.Sin)

                # ---------- phase 1: projections ----------
                with (
                    tc.tile_pool(name="w", bufs=1) as wpool,
                    tc.tile_pool(name="xt", bufs=1) as xtp,
                    tc.tile_pool(name="xin", bufs=2) as xin,
                    tc.tile_pool(name="rope", bufs=4) as rope,
                    tc.tile_pool(name="ps_tr", bufs=2, space="PSUM") as ps_tr,
                    tc.tile_pool(name="ps_mm", bufs=1, space="PSUM") as ps_mm,
                ):
                    self_indent_marker = None  # noqa: F841
                # weights resident: w_all[:, d*1024 + {0:512 wq | 512:768 wk | 768:1024 wv}]
                w_all = wpool.tile([128, DC * 1024], f32r)
                for d in range(DC):
                    rs = slice(d * 128, (d + 1) * 128)
                    nc.sync.dma_start(w_all[:, d * 1024:d * 1024 + 512],
                                      wq_in[rs, :].bitcast(f32r))
                    nc.sync.dma_start(w_all[:, d * 1024 + 512:d * 1024 + 768],
                                      wk_in[rs, :].bitcast(f32r))
                    nc.sync.dma_start(w_all[:, d * 1024 + 768:d * 1024 + 1024],
                                      wv_in[rs, :].bitcast(f32r))

                for tb in range(T // 256):          # 8 t-blocks of 256
                    ts0 = tb * 256
                    # transpose x[t-block] -> xT_buf [128 D, 256 T] per d-chunk
                    xT_buf = xtp.tile([128, DC * 256], f32r, tag="xT")
                    for half in range(2):           # x tile halves [128, 1792]
                        for tc2 in range(2):        # two 128-row t-chunks
                            x_t = xin.tile([128, DC // 2 * 128], f32r,
                                           tag=f"x{tc2}")
                            nc.sync.dma_start(
                                x_t[:],
                                x_in[ts0 + tc2 * 128:ts0 + (tc2 + 1) * 128,
                                     half * 1792:(half + 1) * 1792].bitcast(f32r))
                            for dd_ in range(DC // 2):
                                d = half * (DC // 2) + dd_
                                tp = ps_tr.tile([128, 128], f32r, tag="tr")
                                nc.tensor.transpose(
                                    tp[:], x_t[:, dd_ * 128:(dd_ + 1) * 128],
                                    ident_r[:])
                                nc.vector.tensor_copy(
                                    xT_buf[:, d * 256 + tc2 * 128:
                                           d * 256 + tc2 * 128 + 128], tp[:])

                    # v projection: out [T-part, H-free] — lhsT = xT chunk [D, T128],
                    # rhs = wv [D, 256]; two t-chunks packed 2-up in one bank
                    ps_v = ps_mm.tile([128, 512], f32, tag="psv")
                    for tc2 in range(2):
                        for d in range(DC):
                            nc.tensor.matmul(
                                ps_v[:, tc2 * 256:(tc2 + 1) * 256],
                                xT_buf[:, d * 256 + tc2 * 128:
                                       d * 256 + tc2 * 128 + 128],
                                w_all[:, d * 1024 + 768:d * 1024 + 1024],
                                start=(d == 0), stop=(d == DC - 1))

                    for tc2 in range(2):
                        v_sb = rope.tile([128, 256], f32, tag="v_sb")
                        nc.vector.tensor_copy(v_sb[:],
                                              ps_v[:, tc2 * 256:(tc2 + 1) * 256])
                        nc.sync.dma_start(
                            v_d[ts0 + tc2 * 128:ts0 + (tc2 + 1) * 128, :], v_sb[:])

                    # qT / kT projections + RoPE
                    cos_s = cos_t[:, ts0:ts0 + 256]
                    sin_s = sin_t[:, ts0:ts0 + 256]
                    for pi_, w_off in enumerate((0, 256, 512)):  # q0, q1, k
                        ps_qk = ps_mm.tile([128, 512], f32, tag=f"qk{pi_}")
                        for c in range(2):
                            for d in range(DC):
                                nc.tensor.matmul(
                                    ps_qk[:, c * 256:(c + 1) * 256],
                                    w_all[:, d * 1024 + w_off + c * 128:
                                          d * 1024 + w_off + c * 128 + 128],
                                    xT_buf[:, d * 256:(d + 1) * 256],
                                    start=(d == 0), stop=(d == DC - 1))
                        pA = ps_qk[:, 0:256]
                        pB = ps_qk[:, 256:512]
                        t1 = rope.tile([128, 256], f32, tag="t1")
                        t2 = rope.tile([128, 256], f32, tag="t2")
                        rot0 = rope.tile([128, 256], f32, tag="rot0")
                        rot1 = rope.tile([128, 256], f32, tag="rot1")
                        nc.vector.tensor_tensor(t1[:], pA, cos_s, Alu.mult)
                        nc.vector.tensor_tensor(t2[:], pB, sin_s, Alu.mult)
                        nc.vector.tensor_tensor(rot0[:], t1[:], t2[:], Alu.subtract)
                        nc.vector.tensor_tensor(t1[:], pB, cos_s, Alu.mult)
                        nc.vector.tensor_tensor(t2[:], pA, sin_s, Alu.mult)
                        nc.vector.tensor_tensor(rot1[:], t1[:], t2[:], Alu.add)
                        if pi_ < 2:  # q heads
                            base = pi_ * 256
                            nc.sync.dma_start(qT_d[base:base + 128, ts0:ts0 + 256],
                                              rot0[:])
                            nc.sync.dma_start(qT_d[base + 128:base + 256,
                                                   ts0:ts0 + 256], rot1[:])
                        else:        # k
                            nc.sync.dma_start(kT_d[0:128, ts0:ts0 + 256], rot0[:])
                            nc.sync.dma_start(kT_d[128:256, ts0:ts0 + 256], rot1[:])

            # ---------- phase 2: attention ----------
            with (
                tc.tile_pool(name="kv", bufs=1) as kv,
                tc.tile_pool(name="msk", bufs=1) as msk,
                tc.tile_pool(name="qp", bufs=2) as qp,
                tc.tile_pool(name="pp", bufs=4) as pp,
                tc.tile_pool(name="enc", bufs=2) as encp,
                tc.tile_pool(name="ps_l", bufs=3, space="PSUM") as ps_lp,
                tc.tile_pool(name="ps_e", bufs=1, space="PSUM") as ps_ep,
                tc.tile_pool(name="ps_b", bufs=1, space="PSUM") as ps_bp,
            ):
                # masks (additive, applied pre-softcap)
                mask_tiles = {}
                for dd in CAUSAL_DD:
                    m = msk.tile([128, 512], f32, tag=f"mc{dd}")
                    nc.gpsimd.memset(m[:], 0.0)
                    # live iff i - j + dd <= 0  ⟺  j - i - dd >= 0
                    nc.gpsimd.affine_select(
                        out=m[:], in_=m[:], compare_op=Alu.is_ge, fill=MASK_VAL,
                        base=-dd, pattern=[[1, 512]], channel_multiplier=-1)
                    mask_tiles[dd] = m
                for dd in WINDOW_DD:
                    m = msk.tile([128, 512], f32, tag=f"mw{dd}")
                    nc.gpsimd.memset(m[:], 0.0)
                    nc.gpsimd.affine_select(
                        out=m[:], in_=m[:], compare_op=Alu.is_gt, fill=MASK_VAL,
                        base=dd + WINDOW, pattern=[[-1, 512]], channel_multiplier=1)
                    mask_tiles[dd] = m

                kT_c = []
                for c in range(2):
                    kt = kv.tile([128, T], f32r, tag=f"kt{c}")
                    nc.sync.dma_start(kt[:], kT_d[c * 128:(c + 1) * 128, :].bitcast(f32r))
                    kT_c.append(kt)
                v_all = kv.tile([128, 16 * 256], f32r, tag="v_all")
                for sj in range(16):
                    nc.sync.dma_start(v_all[:, sj * 256:(sj + 1) * 256],
                                      v_d[sj * 128:(sj + 1) * 128, :].bitcast(f32r))

                for lh in range(2):
                    for tb in range(T // 512):
                        js = _live_chunks(tb)
                        q_c = []
                        for c in range(2):
                            qt = qp.tile([128, 512], f32r, tag=f"q{c}")
                            nc.sync.dma_start(
                                qt[:],
                                qT_d[lh * 256 + c * 128:lh * 256 + (c + 1) * 128,
                                     tb * 512:(tb + 1) * 512].bitcast(f32r))
                            q_c.append(qt)
                        e0 = ps_ep.tile([128, 512], f32, tag="e0")
                        e1 = ps_ep.tile([128, 512], f32, tag="e1")
                        den = ps_ep.tile([1, 512], f32, tag="den")
                        for idx, sj in enumerate(js):
                            ps_l = ps_lp.tile([128, 512], f32, tag="l")
                            nc.tensor.matmul(ps_l[:], kT_c[0][:, sj * 128:(sj + 1) * 128],
                                             q_c[0][:], start=True, stop=False)
                            nc.tensor.matmul(ps_l[:], kT_c[1][:, sj * 128:(sj + 1) * 128],
                                             q_c[1][:], start=False, stop=True)
                            dd = sj * 128 - tb * 512
                            if dd in mask_tiles:
                                nc.vector.tensor_tensor(ps_l[:], ps_l[:],
                                                        mask_tiles[dd][:], Alu.add)
                            tmp = pp.tile([128, 512], f32, tag="tmp")
                            nc.scalar.activation(tmp[:], ps_l[:], AF.Tanh,
                                                 scale=TANH_SCALE)
                            pj = pp.tile([128, 512], f32r, tag="pj")
                            nc.scalar.activation(pj[:], tmp[:], AF.Exp,
                                                 scale=SOFT_CAP)
                            first, last = idx == 0, idx == len(js) - 1
                            nc.tensor.matmul(e0[:], v_all[:, sj * 256:sj * 256 + 128],
                                             pj[:], start=first, stop=last)
                            nc.tensor.matmul(e1[:], v_all[:, sj * 256 + 128:sj * 256 + 256],
                                             pj[:], start=first, stop=last)
                            nc.tensor.matmul(den[:], ones_col_r[:], pj[:],
                                             start=first, stop=last)
                        recip = encp.tile([1, 512], f32, tag="recip")
                        nc.vector.reciprocal(recip[:], den[:])
                        bc = ps_bp.tile([128, 512], f32, tag="bc")
                        nc.tensor.matmul(bc[:], ones_row_f[:], recip[:],
                                         start=True, stop=True)
                        bc_sb = encp.tile([128, 512], f32, tag="bc_sb")
                        nc.scalar.copy(bc_sb[:], bc[:])
                        for c, e_ps in enumerate((e0, e1)):
                            e_sb = encp.tile([128, 512], f32, tag=f"e_sb{c}")
                            nc.vector.tensor_tensor(e_sb[:], e_ps[:], bc_sb[:],
                                                    Alu.mult)
                            r0 = c * 128
                            nc.sync.dma_start(cc_in[lh][tb * 2, r0:r0 + 128, :],
                                              e_sb[:, 0:256])
                            nc.sync.dma_start(cc_in[lh][tb * 2 + 1, r0:r0 + 128, :],
                                              e_sb[:, 256:512])

                    nc.gpsimd.collective_compute(
                        "AllToAll", Alu.bypass,
                        replica_groups=[list(range(N_CORES))],
                        ins=[cc_in[lh][:]], outs=[cc_out[lh][:]])

            # ---------- phase 3: output projection for the local T-slice ----------
            with (
                tc.tile_pool(name="ge", bufs=1) as ge,
                tc.tile_pool(name="wo", bufs=2) as wop,
                tc.tile_pool(name="o", bufs=2) as op_,
                tc.tile_pool(name="ps_o", bufs=2, space="PSUM") as ps_op,
            ):
                # per-chunk enc tiles, lh-first order so phase 3 starts
                # right after the first AllToAll; global row = src*512+lh*256+ph*128
                ORDER = [(lh, src, ph) for lh in range(2) for src in range(8)
                         for ph in range(2)]
                enc_c = []
                for i, (lh, src, ph) in enumerate(ORDER):
                    ec = ge.tile([128, 256], f32r, tag=f"e{i}", name=f"e{i}")
                    nc.sync.dma_start(
                        ec[:],
                        cc_out[lh][src, ph * 128:(ph + 1) * 128, :].bitcast(f32r))
                    enc_c.append(ec)
                for db in range(D // 512):
                    wo_buf = wop.tile([128, 32 * 512], f32r, tag="wo")
                    for i, (lh, src, ph) in enumerate(ORDER):
                        grow = (src * 4 + lh * 2 + ph) * 128
                        nc.sync.dma_start(
                            wo_buf[:, i * 512:(i + 1) * 512],
                            wo_in[grow:grow + 128,
                                  db * 512:(db + 1) * 512].bitcast(f32r))
                    for tc2 in range(2):
                        ps_o = ps_op.tile([128, 512], f32, tag="o")
                        for i in range(32):
                            nc.tensor.matmul(
                                ps_o[:],
                                enc_c[i][:, tc2 * 128:(tc2 + 1) * 128],
                                wo_buf[:, i * 512:(i + 1) * 512],
                                start=(i == 0), stop=(i == 31))
                        o_sb = op_.tile([128, 512], f32, tag="o_sb")
                        nc.vector.tensor_copy(o_sb[:], ps_o[:])
                        nc.sync.dma_start(
                            out_ext[tc2 * 128:(tc2 + 1) * 128,
                                    db * 512:(db + 1) * 512], o_sb[:])

    nc.compile()
    return nc


_CACHE = {}
LAST_RESULTS = None


def _get_module():
    if "nc" not in _CACHE:
        _CACHE["nc"] = _build_module()
    return _CACHE["nc"]


def kernel(x, segment_pos, attn_mask, wq, wkv, wo):
    global LAST_RESULTS
    x = np.asarray(x, dtype=np.float32)
    segment_pos = np.asarray(segment_pos, dtype=np.int32)
    wq = np.asarray(wq, dtype=np.float32)
    wkv = np.asarray(wkv, dtype=np.float32)
    wo = np.asarray(wo, dtype=np.float32)

    nc = _get_module()

    consts = np.zeros((128, 130), dtype=np.float32)
    consts[:, 0:128] = np.eye(128, dtype=np.float32)
    consts[:, 128] = 1.0
    consts[:, 129] = (10000.0 ** (-np.arange(128) / 128.0)).astype(np.float32)

    x2d = np.ascontiguousarray(x[0])
    pos = np.ascontiguousarray(segment_pos[0:1])
    wo_flat = np.ascontiguousarray(wo.reshape(4096, D))

    in_maps = []
    for i in range(N_CORES):
        in_maps.append({
            "x": x2d,
            "pos": pos,
            "wq": np.ascontiguousarray(
                np.concatenate([wq[2 * i], wq[2 * i + 1]], axis=1)),
            "wk": np.ascontiguousarray(wkv[0, i]),
            "wv": np.ascontiguousarray(wkv[1, i]),
            "wo": wo_flat,
            "consts": consts,
        })

    LAST_RESULTS = run_bass_kernel_spmd(nc, in_maps,
                                        core_ids=list(range(N_CORES)))
    out = np.concatenate([LAST_RESULTS.results[i]["out"]
                          for i in range(N_CORES)], axis=0)
    return out[None, :, :].astype(np.float32)


# revision 19
# speedup vs baseline: 1.1631x; 1.0038x over previous
"""Trainium2 Bass kernel for sliding-window GQA attention (nn_Attention_12610023981270).

Sharding: 8 cores, head-parallel — core i owns q-heads {2i, 2i+1} and kv-head i
for projections + attention, then an AllToAll switches to sequence-parallel for
the output projection (core i produces output rows [256*i, 256*(i+1))).

Everything on-chip stays "transposed" ([feature, token]) so the only transposes
needed are x itself (PE transpose-mode), and matmuls run in float32r
(full-rate ~1.6e-4 rel-err fp32 mode of the PE).

Model: B=1, T=2048, D=3584, 16 q-heads / 8 kv-heads, head_dim 256,
RoPE, query_scale 1/16, logit softcap 50, causal + sliding window 1024.
"""
import sys

if '/opt/trn_rl_repo' not in sys.path:
    sys.path.insert(0, '/opt/trn_rl_repo')

import numpy as np

import concourse.bass as bass
import concourse.mybir as mybir
import concourse.tile as tile
from concourse import bacc
from concourse.bass_utils import run_bass_kernel_spmd

f32 = mybir.dt.float32
f32r = mybir.dt.float32r
i32 = mybir.dt.int32
AF = mybir.ActivationFunctionType
Alu = mybir.AluOpType

N_CORES = 8
T, D, HD = 2048, 3584, 256
DC = D // 128            # 28 d-chunks
TWO_PI = 6.283185307179586
HALF_PI = 1.5707963267948966
SOFT_CAP = 50.0
QUERY_SCALAR = 0.0625
WINDOW = 1024
MASK_VAL = -1.0e6
TANH_SCALE = QUERY_SCALAR / SOFT_CAP   # folds query scaling into the softcap

# per-tb512 live s-chunks and the additive-mask pattern offsets
CAUSAL_DD = (0, 128, 256, 384)
WINDOW_DD = (-1024, -896, -768, -640)


def _live_chunks(tb):
    t0 = tb * 512
    smin = max(0, t0 - (WINDOW - 1))
    smax = t0 + 511
    return list(range(smin // 128, smax // 128 + 1))


def _build_module():
    nc = bacc.Bacc("TRN2", target_bir_lowering=False, debug=False,
                   num_devices=N_CORES)

    x_in = nc.declare_dram_parameter("x", [T, D], f32, isOutput=False)
    pos_in = nc.declare_dram_parameter("pos", [1, T], i32, isOutput=False)
    wq_in = nc.declare_dram_parameter("wq", [D, 512], f32, isOutput=False)
    wk_in = nc.declare_dram_parameter("wk", [D, 256], f32, isOutput=False)
    wv_in = nc.declare_dram_parameter("wv", [D, 256], f32, isOutput=False)
    wo_in = nc.declare_dram_parameter("wo", [4096, D], f32, isOutput=False)
    # consts: [:, 0:128] identity, [:, 128] ones, [:, 129] inv_timescale
    consts_in = nc.declare_dram_parameter("consts", [128, 130], f32, isOutput=False)
    out_ext = nc.declare_dram_parameter("out", [T // N_CORES, D], f32, isOutput=True)

    qT_d = nc.dram_tensor("qT_d", [512, T], f32)
    kT_d = nc.dram_tensor("kT_d", [256, T], f32)
    v_d = nc.dram_tensor("v_d", [T, 256], f32)
    cc_in = [nc.dram_tensor(f"cc_in{h}", [8, 256, 256], f32) for h in range(2)]
    cc_out = [nc.dram_tensor(f"cc_out{h}", [8, 256, 256], f32) for h in range(2)]

    with tile.TileContext(nc) as tc:
        with tc.tile_pool(name="prep", bufs=1) as prep:
            ident_r = prep.tile([128, 128], f32r)
            nc.sync.dma_start(ident_r[:], consts_in[:, 0:128].bitcast(f32r))
            ones_col_r = prep.tile([128, 1], f32r)
            nc.sync.dma_start(ones_col_r[:], consts_in[:, 128:129].bitcast(f32r))
            ones_row_f = prep.tile([1, 128], f32)
            nc.sync.dma_start(ones_row_f[:],
                              consts_in[:, 128:129].rearrange("p one -> one p"))
            ones_row_r = prep.tile([1, 128], f32r)
            nc.sync.dma_start(
                ones_row_r[:],
                consts_in[:, 128:129].rearrange("p one -> one p").bitcast(f32r))
            inv_ts = prep.tile([128, 1], f32)
            nc.sync.dma_start(inv_ts[:], consts_in[:, 129:130])

            # ---------- phase 0: RoPE sin/cos tables [128, T] ----------
            with tc.tile_pool(name="tables", bufs=1) as tbl:
                sin_t = tbl.tile([128, T], f32)
                cos_t = tbl.tile([128, T], f32)
                with (
                    tc.tile_pool(name="p0", bufs=1) as p0,
                    tc.tile_pool(name="ps0", bufs=2, space="PSUM") as ps0,
                ):
                    pos_i = p0.tile([1, T], i32)
                    nc.sync.dma_start(pos_i[:], pos_in[:])
                    pos_f = p0.tile([1, T], f32)
                    nc.vector.tensor_copy(pos_f[:], pos_i[:])
                    theta = p0.tile([128, T], f32)
                    for b in range(T // 512):
                        ps = ps0.tile([128, 512], f32, tag="bc0")
                        nc.tensor.matmul(ps[:], ones_row_f[:],
                                         pos_f[:, b * 512:(b + 1) * 512],
                                         start=True, stop=True)
                        nc.vector.tensor_scalar(theta[:, b * 512:(b + 1) * 512],
                                                ps[:], inv_ts[:], None, Alu.mult)

                    def range_reduce(dst, pre_add):
                        u = p0.tile([128, T], f32, tag="rr_u")
                        nc.vector.tensor_scalar(u[:], theta[:], pre_add,
                                                1.0 / TWO_PI, Alu.add, Alu.mult)
                        k_i = p0.tile([128, T], i32, tag="rr_k")
                        nc.vector.tensor_copy(k_i[:], u[:])
                        k_f = p0.tile([128, T], f32, tag="rr_kf")
                        nc.vector.tensor_copy(k_f[:], k_i[:])
                        r = p0.tile([128, T], f32, tag="rr_r")
                        nc.vector.tensor_tensor(r[:], u[:], k_f[:], Alu.subtract)
                        nc.vector.tensor_scalar(dst[:], r[:], TWO_PI, None,
                                                Alu.mult)

                    th_r = p0.tile([128, T], f32, tag="th_r")
                    range_reduce(th_r, 0.0)
                    nc.scalar.activation(sin_t[:], th_r[:], AF.Sin)
                    th_r2 = p0.tile([128, T], f32, tag="th_r")
                    range_reduce(th_r2, HALF_PI)
                    nc.scalar.activation(cos_t[:], th_r2[:], AF.Sin)

                # ---------- phase 1: projections ----------
                with (
                    tc.tile_pool(name="w", bufs=1) as wpool,
                    tc.tile_pool(name="xt", bufs=1) as xtp,
                    tc.tile_pool(name="xin", bufs=2) as xin,
                    tc.tile_pool(name="rope", bufs=4) as rope,
                    tc.tile_pool(name="ps_tr", bufs=2, space="PSUM") as ps_tr,
                    tc.tile_pool(name="ps_mm", bufs=1, space="PSUM") as ps_mm,
                ):
                    self_indent_marker = None  # noqa: F841
                # weights resident: w_all[:, d*1024 + {0:512 wq | 512:768 wk | 768:1024 wv}]
                w_all = wpool.tile([128, DC * 1024], f32r)
                for d in range(DC):
                    rs = slice(d * 128, (d + 1) * 128)
                    nc.sync.dma_start(w_all[:, d * 1024:d * 1024 + 512],
                                      wq_in[rs, :].bitcast(f32r))
                    nc.sync.dma_start(w_all[:, d * 1024 + 512:d * 1024 + 768],
                                      wk_in[rs, :].bitcast(f32r))
                    nc.sync.dma_start(w_all[:, d * 1024 + 768:d * 1024 + 1024],
                                      wv_in[rs, :].bitcast(f32r))

                for tb in range(T // 256):          # 8 t-blocks of 256
                    ts0 = tb * 256
                    # transpose x[t-block] -> xT_buf [128 D, 256 T] per d-chunk
                    xT_buf = xtp.tile([128, DC * 256], f32r, tag="xT")
                    for half in range(2):           # x tile halves [128, 1792]
                        for tc2 in range(2):        # two 128-row t-chunks
                            x_t = xin.tile([128, DC // 2 * 128], f32r,
                                           tag=f"x{tc2}")
                            nc.sync.dma_start(
                                x_t[:],
                                x_in[ts0 + tc2 * 128:ts0 + (tc2 + 1) * 128,
                                     half * 1792:(half + 1) * 1792].bitcast(f32r))
                            for dd_ in range(DC // 2):
                                d = half * (DC // 2) + dd_
                                tp = ps_tr.tile([128, 128], f32r, tag="tr")
                                nc.tensor.transpose(
                                    tp[:], x_t[:, dd_ * 128:(dd_ + 1) * 128],
                                    ident_r[:])
                                nc.vector.tensor_copy(
                                    xT_buf[:, d * 256 + tc2 * 128:
                                           d * 256 + tc2 * 128 + 128], tp[:])

                    # v projection: out [T-part, H-free] — lhsT = xT chunk [D, T128],
                    # rhs = wv [D, 256]; two t-chunks packed 2-up in one bank
                    ps_v = ps_mm.tile([128, 512], f32, tag="psv")
                    for tc2 in range(2):
                        for d in range(DC):
                            nc.tensor.matmul(
                                ps_v[:, tc2 * 256:(tc2 + 1) * 256],
                                xT_buf[:, d * 256 + tc2 * 128:
                                       d * 256 + tc2 * 128 + 128],
                                w_all[:, d * 1024 + 768:d * 1024 + 1024],
                                start=(d == 0), stop=(d == DC - 1))

                    for tc2 in range(2):
                        v_sb = rope.tile([128, 256], f32, tag="v_sb")
                        nc.vector.tensor_copy(v_sb[:],
                                              ps_v[:, tc2 * 256:(tc2 + 1) * 256])
                        nc.sync.dma_start(
                            v_d[ts0 + tc2 * 128:ts0 + (tc2 + 1) * 128, :], v_sb[:])

                    # qT / kT projections + RoPE
                    cos_s = cos_t[:, ts0:ts0 + 256]
                    sin_s = sin_t[:, ts0:ts0 + 256]
                    for pi_, w_off in enumerate((0, 256, 512)):  # q0, q1, k
                        ps_qk = ps_mm.tile([128, 512], f32, tag=f"qk{pi_}")
                        for c in range(2):
                            for d in range(DC):
                                nc.tensor.matmul(
                                    ps_qk[:, c * 256:(c + 1) * 256],
                                    w_all[:, d * 1024 + w_off + c * 128:
                                          d * 1024 + w_off + c * 128 + 128],
                                    xT_buf[:, d * 256:(d + 1) * 256],
                                    start=(d == 0), stop=(d == DC - 1))
                        pA = ps_qk[:, 0:256]
                        pB = ps_qk[:, 256:512]
                        t1 = rope.tile([128, 256], f32, tag="t1")
                        t2 = rope.tile([128, 256], f32, tag="t2")
                        rot0 = rope.tile([128, 256], f32, tag="rot0")
                        rot1 = rope.tile([128, 256], f32, tag="rot1")
                        nc.vector.tensor_tensor(t1[:], pA, cos_s, Alu.mult)
                        nc.vector.tensor_tensor(t2[:], pB, sin_s, Alu.mult)
                        nc.vector.tensor_tensor(rot0[:], t1[:], t2[:], Alu.subtract)
                        nc.vector.tensor_tensor(t1[:], pB, cos_s, Alu.mult)
                        nc.vector.tensor_tensor(t2[:], pA, sin_s, Alu.mult)
                        nc.vector.tensor_tensor(rot1[:], t1[:], t2[:], Alu.add)
                        if pi_ < 2:  # q heads
                            base = pi_ * 256
                            nc.sync.dma_start(qT_d[base:base + 128, ts0:ts0 + 256],
                                              rot0[:])
                            nc.sync.dma_start(qT_d[base + 128:base + 256,
                                                   ts0:ts0 + 256], rot1[:])
                        else:        # k
                            nc.sync.dma_start(kT_d[0:128, ts0:ts0 + 256], rot0[:])
                            nc.sync.dma_start(kT_d[128:256, ts0:ts0 + 256], rot1[:])

            # ---------- phase 2: attention ----------
            with (
                tc.tile_pool(name="kv", bufs=1) as kv,
                tc.tile_pool(name="msk", bufs=1) as msk,
                tc.tile_pool(name="qp", bufs=2) as qp,
                tc.tile_pool(name="pp", bufs=4) as pp,
                tc.tile_pool(name="enc", bufs=2) as encp,
                tc.tile_pool(name="ps_l", bufs=3, space="PSUM") as ps_lp,
                tc.tile_pool(name="ps_e", bufs=1, space="PSUM") as ps_ep,
                tc.tile_pool(name="ps_b", bufs=1, space="PSUM") as ps_bp,
            ):
                # masks (additive, applied pre-softcap)
                mask_tiles = {}
                for dd in CAUSAL_DD:
                    m = msk.tile([128, 512], f32, tag=f"mc{dd}")
                    nc.gpsimd.memset(m[:], 0.0)
                    # live iff i - j + dd <= 0  ⟺  j - i - dd >= 0
                    nc.gpsimd.affine_select(
                        out=m[:], in_=m[:], compare_op=Alu.is_ge, fill=MASK_VAL,
                        base=-dd, pattern=[[1, 512]], channel_multiplier=-1)
                    mask_tiles[dd] = m
                for dd in WINDOW_DD:
                    m = msk.tile([128, 512], f32, tag=f"mw{dd}")
                    nc.gpsimd.memset(m[:], 0.0)
                    nc.gpsimd.affine_select(
                        out=m[:], in_=m[:], compare_op=Alu.is_gt, fill=MASK_VAL,
                        base=dd + WINDOW, pattern=[[-1, 512]], channel_multiplier=1)
                    mask_tiles[dd] = m

                kT_c = []
                for c in range(2):
                    kt = kv.tile([128, T], f32r, tag=f"kt{c}")
                    nc.sync.dma_start(kt[:], kT_d[c * 128:(c + 1) * 128, :].bitcast(f32r))
                    kT_c.append(kt)
                v_all = kv.tile([128, 16 * 256], f32r, tag="v_all")
                for sj in range(16):
                    nc.sync.dma_start(v_all[:, sj * 256:(sj + 1) * 256],
                                      v_d[sj * 128:(sj + 1) * 128, :].bitcast(f32r))

                for lh in range(2):
                    for tb in range(T // 512):
                        js = _live_chunks(tb)
                        q_c = []
                        for c in range(2):
                            qt = qp.tile([128, 512], f32r, tag=f"q{c}")
                            nc.sync.dma_start(
                                qt[:],
                                qT_d[lh * 256 + c * 128:lh * 256 + (c + 1) * 128,
                                     tb * 512:(tb + 1) * 512].bitcast(f32r))
                            q_c.append(qt)
                        e0 = ps_ep.tile([128, 512], f32, tag="e0")
                        e1 = ps_ep.tile([128, 512], f32, tag="e1")
                        den = ps_ep.tile([1, 512], f32, tag="den")
                        for idx, sj in enumerate(js):
                            ps_l = ps_lp.tile([128, 512], f32, tag="l")
                            nc.tensor.matmul(ps_l[:], kT_c[0][:, sj * 128:(sj + 1) * 128],
                                             q_c[0][:], start=True, stop=False)
                            nc.tensor.matmul(ps_l[:], kT_c[1][:, sj * 128:(sj + 1) * 128],
                                             q_c[1][:], start=False, stop=True)
                            dd = sj * 128 - tb * 512
                            if dd in mask_tiles:
                                nc.vector.tensor_tensor(ps_l[:], ps_l[:],
                                                        mask_tiles[dd][:], Alu.add)
                            tmp = pp.tile([128, 512], f32, tag="tmp")
                            nc.scalar.activation(tmp[:], ps_l[:], AF.Tanh,
                                                 scale=TANH_SCALE)
                            pj = pp.tile([128, 512], f32r, tag="pj")
                            nc.scalar.activation(pj[:], tmp[:], AF.Exp,
                                                 scale=SOFT_CAP)
                            first, last = idx == 0, idx == len(js) - 1
                            nc.tensor.matmul(e0[:], v_all[:, sj * 256:sj * 256 + 128],
                                             pj[:], start=first, stop=last)
                            nc.tensor.matmul(e1[:], v_all[:, sj * 256 + 128:sj * 256 + 256],
                                             pj[:], start=first, stop=last)
                            nc.tensor.matmul(den[:], ones_col_r[:], pj[:],
                                             start=first, stop=last)
                        recip = encp.tile([1, 512], f32r, tag="recip")
                        with nc.allow_low_precision(reason="f32r recip bcast"):
                            nc.vector.reciprocal(recip[:], den[:])
                        bc = ps_bp.tile([128, 512], f32, tag="bc")
                        nc.tensor.matmul(bc[:], ones_row_r[:], recip[:],
                                         start=True, stop=True)
                        bc_sb = encp.tile([128, 512], f32, tag="bc_sb")
                        nc.scalar.copy(bc_sb[:], bc[:])
                        for c, e_ps in enumerate((e0, e1)):
                            e_sb = encp.tile([128, 512], f32, tag=f"e_sb{c}")
                            nc.vector.tensor_tensor(e_sb[:], e_ps[:], bc_sb[:],
                                                    Alu.mult)
                            r0 = c * 128
                            nc.sync.dma_start(cc_in[lh][tb * 2, r0:r0 + 128, :],
                                              e_sb[:, 0:256])
                            nc.sync.dma_start(cc_in[lh][tb * 2 + 1, r0:r0 + 128, :],
                                              e_sb[:, 256:512])

                    nc.gpsimd.collective_compute(
                        "AllToAll", Alu.bypass,
                        replica_groups=[list(range(N_CORES))],
                        ins=[cc_in[lh][:]], outs=[cc_out[lh][:]])

            # ---------- phase 3: output projection for the local T-slice ----------
            with (
                tc.tile_pool(name="ge", bufs=1) as ge,
                tc.tile_pool(name="wo", bufs=2) as wop,
                tc.tile_pool(name="o", bufs=2) as op_,
                tc.tile_pool(name="ps_o", bufs=4, space="PSUM") as ps_op,
            ):
                # per-chunk enc tiles, lh-first order so phase 3 starts
                # right after the first AllToAll; global row = src*512+lh*256+ph*128
                ORDER = [(lh, src, ph) for lh in range(2) for src in range(8)
                         for ph in range(2)]
                enc_c = []
                for i, (lh, src, ph) in enumerate(ORDER):
                    ec = ge.tile([128, 256], f32r, tag=f"e{i}", name=f"e{i}")
                    nc.sync.dma_start(
                        ec[:],
                        cc_out[lh][src, ph * 128:(ph + 1) * 128, :].bitcast(f32r))
                    enc_c.append(ec)
                for db in range(D // 512):
                    wo_buf = wop.tile([128, 32 * 512], f32r, tag="wo")
                    for i, (lh, src, ph) in enumerate(ORDER):
                        grow = (src * 4 + lh * 2 + ph) * 128
                        nc.sync.dma_start(
                            wo_buf[:, i * 512:(i + 1) * 512],
                            wo_in[grow:grow + 128,
                                  db * 512:(db + 1) * 512].bitcast(f32r))
                    for tc2 in range(2):
                        ps_o = ps_op.tile([128, 512], f32, tag="o")
                        for i in range(32):
                            nc.tensor.matmul(
                                ps_o[:],
                                enc_c[i][:, tc2 * 128:(tc2 + 1) * 128],
                                wo_buf[:, i * 512:(i + 1) * 512],
                                start=(i == 0), stop=(i == 31))
                        o_sb = op_.tile([128, 512], f32, tag="o_sb")
                        nc.vector.tensor_copy(o_sb[:], ps_o[:])
                        nc.sync.dma_start(
                            out_ext[tc2 * 128:(tc2 + 1) * 128,
                                    db * 512:(db + 1) * 512], o_sb[:])

    nc.compile()
    return nc


_CACHE = {}
LAST_RESULTS = None


def _get_module():
    if "nc" not in _CACHE:
        _CACHE["nc"] = _build_module()
    return _CACHE["nc"]


def kernel(x, segment_pos, attn_mask, wq, wkv, wo):
    global LAST_RESULTS
    x = np.asarray(x, dtype=np.float32)
    segment_pos = np.asarray(segment_pos, dtype=np.int32)
    wq = np.asarray(wq, dtype=np.float32)
    wkv = np.asarray(wkv, dtype=np.float32)
    wo = np.asarray(wo, dtype=np.float32)

    nc = _get_module()

    consts = np.zeros((128, 130), dtype=np.float32)
    consts[:, 0:128] = np.eye(128, dtype=np.float32)
    consts[:, 128] = 1.0
    consts[:, 129] = (10000.0 ** (-np.arange(128) / 128.0)).astype(np.float32)

    x2d = np.ascontiguousarray(x[0])
    pos = np.ascontiguousarray(segment_pos[0:1])
    wo_flat = np.ascontiguousarray(wo.reshape(4096, D))

    in_maps = []
    for i in range(N_CORES):
        in_maps.append({
            "x": x2d,
            "pos": pos,
            "wq": np.ascontiguousarray(
                np.concatenate([wq[2 * i], wq[2 * i + 1]], axis=1)),
            "wk": np.ascontiguousarray(wkv[0, i]),
            "wv": np.ascontiguousarray(wkv[1, i]),
            "wo": wo_flat,
            "consts": consts,
        })

    LAST_RESULTS = run_bass_kernel_spmd(nc, in_maps,
                                        core_ids=list(range(N_CORES)))
    out = np.concatenate([LAST_RESULTS.results[i]["out"]
                          for i in range(N_CORES)], axis=0)
    return out[None, :, :].astype(np.float32)
